# revision 1
# baseline (speedup 1.0000x reference)
"""Trainium2 Bass kernel for nn_ODE_71743133713072 (v2).

Semantics (unrolled from the reference lax.scan):
  out[:, 0]   = lat[:, 0]                       (host)
  out[:, 2]   = lat[:, 1]                       (host; the scan's dt=0 step)
  out[:, t+1] = lat[:, t] + h*f(lat[:, t])      t = 0..99   (parallel part)
  out[:, k+1] = y += h*f(y), y0 = out[:, 100]   k = 100..118 (serial chain)
with f the D->U->U->D tanh MLP, h = ts[1]-ts[0].

Key design points (all validated on hw in micro benchmarks):
  - Batch 1024 split over 8 cores (128 rows = partition width per core).
  - x^T for layer 1 arrives via hardware DMA-transpose of an fp8 copy of
    the latents packed as uint16 byte-pairs; DoubleRow fp8 matmuls consume
    the pairs through a byte-interleaved access pattern (k = 2p+j) with
    host-permuted W1 rows. No on-chip transpose/cast of x at all.
  - h*b3 is folded into the natural-layout euler operand on the host
    (latb = bf16(lat + h*b3)), so layer 3 needs no bias seed matmul.
  - b1/b2 activation biases use a pairing permutation: U features are
    sorted by bias so the two features sharing an SBUF partition have
    nearly equal bias; one [p,1] bias AP then covers a whole layer ->
    a single free-1024 activation instruction per layer per group.
  - Outputs are stored bf16 (t=1..100) / f32-transposed (chain) and
    assembled to f32 on the host; rel-err budget is ~2e-3 vs 2e-2 gate.
  - DMA issue is split between the Sync HWDGE queue (transposed loads,
    bf16 stores) and the GpSimd SWDGE queue (natural loads, chain stores).
"""

import os
import sys
from contextlib import ExitStack

import numpy as np

for _p in ("/opt/trn_rl_repo", "/root/.axon_site/_ro/trn_rl_repo"):
    if os.path.isdir(_p) and _p not in sys.path:
        sys.path.append(_p)

import ml_dtypes  # noqa: E402

B, T_OBS, KPRED, D = 1024, 100, 20, 256
T = T_OBS + KPRED          # 120
NCORES = 8
PB = B // NCORES           # 128 rows per core
P = 128
G = 4                      # time steps per compute group
NG = T_OBS // G            # 25 groups


def _emit(ctx, tc, t_lat8u, t_latb, t_w8b, t_hbo, t_f32b,
          t_out16, t_outch, h):
    import concourse.mybir as mybir

    nc = tc.nc
    F32 = mybir.dt.float32
    BF16 = mybir.dt.bfloat16
    FP8 = mybir.dt.float8e4
    AF = mybir.ActivationFunctionType
    ALU = mybir.AluOpType
    DR = mybir.MatmulPerfMode.DoubleRow

    h8 = float(h / 8.0)

    const = ctx.enter_context(tc.tile_pool(name="const", bufs=1))
    # fp8 blob: w1i | w2i | w3m | w1c along the last axis
    w8b = const.tile([P, 2, 4, 2 * P], FP8, tag="w8b")
    w1i, w2i, w3m, w1c = (w8b[:, :, i, :] for i in range(4))
    # f32 blob: id32 | bs
    f32b = const.tile([P, P + 2], F32, tag="f32b")
    id32 = f32b[:, 0:P]
    bs = f32b[:, P:P + 2]
    hbo = const.tile([1, 3 * P], BF16, tag="hbo")   # 8*b3(256) | ones(128)
    b3s8 = hbo[:, 0:2 * P]
    ones = hbo[:, 2 * P:3 * P]
    chainbuf = const.tile([P, KPRED - 1, 2, P], F32, tag="chainbuf")

    def load_consts_main():
        # gpsimd SWDGE queue: its preamble ends earlier than Sync's and it
        # has no other early work, so weights land before the first
        # transpose finishes. w1i first — it alone gates the first matmul.
        nc.gpsimd.dma_start(w8b[:, :, 0, :], t_w8b[:, :, 0, :])
        nc.gpsimd.dma_start(f32b[:], t_f32b[:])
        nc.gpsimd.dma_start(w8b[:, :, 1:, :], t_w8b[:, :, 1:, :])

    def load_consts_chain():
        nc.gpsimd.dma_start(hbo[:], t_hbo[:])

    b1s = bs[:, 0:1]
    b2s = bs[:, 1:2]

    # warm the Tanh activation table during the startup DMA window so the
    # 1.3us ACT_TABLE_LOAD doesn't gate the first real activation
    warm = const.tile([1, 2], F32, tag="warm")
    nc.vector.memset(warm[:, 0:1], 0.0)
    nc.scalar.activation(warm[:, 1:2], warm[:, 0:1], AF.Tanh)

    xtsp = ctx.enter_context(tc.tile_pool(name="xts", bufs=4))
    x16p = ctx.enter_context(tc.tile_pool(name="x16", bufs=4))
    h1p = ctx.enter_context(tc.tile_pool(name="h1", bufs=3))
    h2p = ctx.enter_context(tc.tile_pool(name="h2", bufs=3))
    o16p = ctx.enter_context(tc.tile_pool(name="o16", bufs=4))
    chsb = ctx.enter_context(tc.tile_pool(name="chsb", bufs=6))

    mmp = ctx.enter_context(tc.tile_pool(name="mmp", bufs=2, space="PSUM"))
    fnp = ctx.enter_context(tc.tile_pool(name="fnp", bufs=2, space="PSUM"))
    chp = ctx.enter_context(tc.tile_pool(name="chp", bufs=2, space="PSUM"))

    def stage_load(t0, nt):
        """one transposed + one natural load covering nt timesteps at t0."""
        xts = xtsp.tile([P, nt, P], BF16, tag="xts")
        nc.sync.dma_start_transpose(
            xts[:], t_lat8u[:, t0:t0 + nt, :].rearrange("p a b -> p (a b)"))
        x16 = x16p.tile([P, nt, 2 * P], BF16, tag="x16")
        nc.sync.dma_start(x16[:], t_latb[:, t0:t0 + nt, :])
        return xts, x16

    def stage_l1(xts_ap, half):
        # interleaved fp8 view: [p, j, (t b)], k = 2p + j
        rhs1 = xts_ap.bitcast(FP8).rearrange(
            "p t (b j) -> p j (t b)", j=2)[:, :, half * G * P:(half + 1) * G * P]
        mm = mmp.tile([P, 2, G * P], F32, tag="mm", name="l1")
        for mc in range(2):
            nc.tensor.matmul(mm[:, mc, :], w1i[:, :, mc * P:(mc + 1) * P],
                             rhs1, start=True, stop=True, perf_mode=DR)
        return mm

    def stage_h1(mm):
        h1 = h1p.tile([P, 2, G * P], FP8, tag="h1")
        nc.scalar.activation(h1[:].rearrange("p a b -> p (a b)"),
                             mm[:].rearrange("p a b -> p (a b)"),
                             AF.Tanh, bias=b1s, scale=0.125)
        return h1

    def stage_l2(h1):
        mm2 = mmp.tile([P, 2, G * P], F32, tag="mm", name="l2")
        for mc in range(2):
            nc.tensor.matmul(mm2[:, mc, :], w2i[:, :, mc * P:(mc + 1) * P],
                             h1[:], start=True, stop=True, perf_mode=DR)
        return mm2

    def stage_h2(mm2):
        h2 = h2p.tile([P, 2, G * P], FP8, tag="h2")
        nc.scalar.activation(h2[:].rearrange("p a b -> p (a b)"),
                             mm2[:].rearrange("p a b -> p (a b)"),
                             AF.Tanh, bias=b2s, scale=0.125)
        return h2

    def stage_fn(h2, x16, xoff, o16, ooff, want_o32=False):
        """L3 + euler for one group; o16/x16 may be pair-sized tiles."""
        o32 = None
        for half in ([1, 0] if want_o32 else [0, 1]):
            fn = fnp.tile([P, 2, 2 * P], F32, tag="fn")
            for i in range(2):
                tt = 2 * half + i
                nc.tensor.matmul(fn[:, i, :], h2[:, :, tt * P:(tt + 1) * P],
                                 w3m[:], start=True, stop=True, perf_mode=DR)
            xs = xoff + 2 * half
            os = ooff + 2 * half
            nc.vector.scalar_tensor_tensor(
                o16[:, os:os + 2, :].rearrange("p a b -> p (a b)"),
                fn[:].rearrange("p a b -> p (a b)"), h8,
                x16[:, xs:xs + 2, :].rearrange("p a b -> p (a b)"),
                ALU.mult, ALU.add)
            if want_o32 and half == 1:
                o32 = chsb.tile([P, 2, 2 * P], F32, tag="o32")
                nc.vector.scalar_tensor_tensor(
                    o32[:].rearrange("p a b -> p (a b)"),
                    fn[:].rearrange("p a b -> p (a b)"), h8,
                    x16[:, xs:xs + 2, :].rearrange("p a b -> p (a b)"),
                    ALU.mult, ALU.add)
        return o32

    def chain_init(o32):
        # y0 = out[:, 100] = o32[:, 1, :]; carry is y^T f32 [p(d), dc, b]
        y0p = chp.tile([P, 2, P], F32, tag="ch", name="y0p")
        for dc in range(2):
            nc.tensor.transpose(y0p[:, dc, :],
                                o32[:, 1, dc * P:(dc + 1) * P], id32[:])
        yt = chsb.tile([P, 2, P], F32, tag="yt")
        nc.vector.tensor_copy(yt[:], y0p[:])
        y8 = chsb.tile([P, 2, P], FP8, tag="y8")
        nc.vector.tensor_copy(y8[:], yt[:])
        return yt, y8

    # chain sub-steps: each is one engine-hop bundle so the PE never
    # head-of-line-stalls waiting for an activation of the same step
    def chain_sub1(y8):
        c1 = chp.tile([P, 2, P], F32, tag="ch", name="c1")
        for mc in range(2):
            nc.tensor.matmul(c1[:, mc, :], w1c[:, :, mc * P:(mc + 1) * P],
                             y8[:], start=True, stop=True, perf_mode=DR)
        c1s = chsb.tile([P, 2, P], FP8, tag="c1s")
        nc.scalar.activation(c1s[:].rearrange("p a b -> p (a b)"),
                             c1[:].rearrange("p a b -> p (a b)"),
                             AF.Tanh, bias=b1s, scale=0.125)
        return c1s

    def chain_sub2(c1s):
        c2 = chp.tile([P, 2, P], F32, tag="ch", name="c2")
        for mc in range(2):
            nc.tensor.matmul(c2[:, mc, :], w2i[:, :, mc * P:(mc + 1) * P],
                             c1s[:], start=True, stop=True, perf_mode=DR)
        c2s = chsb.tile([P, 2, P], FP8, tag="c2s")
        nc.scalar.activation(c2s[:].rearrange("p a b -> p (a b)"),
                             c2[:].rearrange("p a b -> p (a b)"),
                             AF.Tanh, bias=b2s, scale=0.125)
        return c2s

    def chain_sub3(k, c2s, yt):
        # c3 = 8*(W3^T c2s + b3) via fp8 DR (w3m doubles as lhsT here);
        # the h/8 scale is folded into the closing STT
        c3 = chp.tile([P, 2, P], F32, tag="ch", name="c3")
        for mc in range(2):
            nc.tensor.matmul(c3[:, mc, :], b3s8[:, mc * P:(mc + 1) * P],
                             ones[:], start=True, stop=False)
            nc.tensor.matmul(c3[:, mc, :], w3m[:, :, mc * P:(mc + 1) * P],
                             c2s[:], start=False, stop=True, perf_mode=DR)
        ytn = chainbuf[:, k, :, :]
        nc.vector.scalar_tensor_tensor(
            ytn.rearrange("p a b -> p (a b)"),
            c3[:].rearrange("p a b -> p (a b)"), h8,
            yt[:].rearrange("p a b -> p (a b)"), ALU.mult, ALU.add)
        y8 = None
        if k + 1 < KPRED - 1:
            y8 = chsb.tile([P, 2, P], FP8, tag="y8")
            nc.vector.tensor_copy(y8[:], ytn)
        return ytn, y8

    # --- emission ---
    # Pair slots of 2 groups with stage interleaving (fills the in-order
    # Act queue's l2-wait gap with the other group's activation). The first
    # pair is (g24, g23) so the chain can start immediately after it; then
    # (g0,g1)..(g20,g21), then g22 alone. 2 chain steps are woven into each
    # slot, spread across it. Loads on the Sync HWDGE queue (first loads
    # issued before const loads), stores on the GpSimd SWDGE queue.
    NCH = KPRED - 1  # 19 chain steps
    state = dict(yt=None, y8=None, c1s=None, c2s=None, ph=0, ch=0, flushed=0)

    def flush_chain(upto):
        nc.gpsimd.dma_start(
            t_outch[state["flushed"]:upto].rearrange("k p a b -> p k a b"),
            chainbuf[:, state["flushed"]:upto, :, :])
        state["flushed"] = upto

    def chain_tick():
        if state["yt"] is None or state["ch"] >= NCH:
            return
        if state["ph"] == 0:
            state["c1s"] = chain_sub1(state["y8"])
            state["ph"] = 1
        elif state["ph"] == 1:
            state["c2s"] = chain_sub2(state["c1s"])
            state["ph"] = 2
        else:
            ytn, y8 = chain_sub3(state["ch"], state["c2s"], state["yt"])
            state["yt"], state["y8"] = ytn, y8
            state["ph"] = 0
            state["ch"] += 1
            if state["ch"] in (4, 8, 12, 16, NCH):
                flush_chain(state["ch"])

    def do_pair(t0, xts, x16, first_is_b=False, want_o32=False,
                nhalves=2):
        halves = [1, 0] if first_is_b else list(range(nhalves))
        mms = {h: stage_l1(xts, h) for h in halves}
        h1s = {}
        for h in halves:
            h1s[h] = stage_h1(mms[h])
            chain_tick()
        mm2s = {}
        for h in halves:
            mm2s[h] = stage_l2(h1s[h])
            chain_tick()
        h2s = {}
        for h in halves:
            h2s[h] = stage_h2(mm2s[h])
            chain_tick()
        nt = len(halves) * G
        o16 = o16p.tile([P, nt, 2 * P], BF16, tag="o16")
        o32 = None
        for h in halves:
            r = stage_fn(h2s[h], x16, h * G, o16, h * G,
                         want_o32=want_o32 and h == (1 if first_is_b else 0))
            if r is not None:
                o32 = r
            chain_tick()
            if len(halves) > 2:
                # last (triple) slot: flush each group as it completes so
                # only the final group's store tails the run
                nc.gpsimd.dma_start(t_out16[:, t0 + h * G:t0 + (h + 1) * G, :],
                                    o16[:, h * G:(h + 1) * G, :])
        if len(halves) <= 2:
            nc.gpsimd.dma_start(t_out16[:, t0:t0 + nt, :], o16[:])
        return o32

    # pair 0: groups 23 (half0) + 24 (half1), g24 first. Weights go first
    # on the DMA queue so LDWEIGHTS completes during the transpose
    # transfer; the natural load is only needed at the euler stage.
    load_consts_main()
    xts0 = xtsp.tile([P, 2 * G, P], BF16, tag="xts")
    nc.sync.dma_start_transpose(
        xts0[:], t_lat8u[:, (NG - 2) * G:NG * G, :].rearrange("p a b -> p (a b)"))
    loads = {1: stage_load(0, 2 * G)}
    x160 = x16p.tile([P, 2 * G, 2 * P], BF16, tag="x16")
    nc.sync.dma_start(x160[:], t_latb[:, (NG - 2) * G:NG * G, :])
    load_consts_chain()
    o32 = do_pair((NG - 2) * G, xts0[:], x160[:], first_is_b=True,
                  want_o32=True)
    state["yt"], state["y8"] = chain_init(o32)
    # kick the first chain step's matmul+act immediately — the engines are
    # otherwise idle right after pair 0 drains
    chain_tick()
    chain_tick()

    # 11 slots: pairs (0,1),...,(18,19) then the (20,21,22) triple
    slots = [(2 * i * G, 2) for i in range(10)] + [(20 * G, 3)]
    for si in range(len(slots)):
        t0, nh = slots[si]
        xts, x16 = loads[si + 1]
        if si + 1 < len(slots):
            nt0, nnh = slots[si + 1]
            loads[si + 2] = stage_load(nt0, nnh * G)
        do_pair(t0, xts[:], x16[:], nhalves=nh)
        del loads[si + 1]
    while state["ch"] < NCH:
        chain_tick()
    if state["flushed"] < NCH:
        flush_chain(NCH)


def _build(h):
    import concourse.mybir as mybir
    import concourse.tile as tile
    from concourse import bacc

    F32 = mybir.dt.float32
    BF16 = mybir.dt.bfloat16
    FP8 = mybir.dt.float8e4

    nc = bacc.Bacc("TRN2", target_bir_lowering=False, debug=False,
                   num_devices=NCORES)
    t_lat8u = nc.dram_tensor("lat8u", [PB, T_OBS, P], BF16,
                             kind="ExternalInput").ap()
    t_latb = nc.dram_tensor("latb", [PB, T_OBS, D], BF16,
                            kind="ExternalInput").ap()
    t_w8b = nc.dram_tensor("w8b", [P, 2, 4, D], FP8, kind="ExternalInput").ap()
    t_hbo = nc.dram_tensor("hbo", [1, D + P], BF16,
                           kind="ExternalInput").ap()
    t_f32b = nc.dram_tensor("f32b", [P, P + 2], F32,
                            kind="ExternalInput").ap()
    t_out16 = nc.dram_tensor("out16", [PB, T_OBS, D], BF16,
                             kind="ExternalOutput").ap()
    t_outch = nc.dram_tensor("outch", [KPRED - 1, P, 2, P], F32,
                             kind="ExternalOutput").ap()

    with tile.TileContext(nc) as tc, ExitStack() as ctx:
        _emit(ctx, tc, t_lat8u, t_latb, t_w8b, t_hbo, t_f32b,
              t_out16, t_outch, h)
    nc.compile()
    return nc


def _host_inputs(inputs):
    ts = np.asarray(inputs["time_steps"], np.float32)
    h = float(np.float32(ts[1]) - np.float32(ts[0]))

    bf = ml_dtypes.bfloat16
    f8 = ml_dtypes.float8_e4m3

    W1 = np.asarray(inputs["W1"], np.float32)
    W2 = np.asarray(inputs["W2"], np.float32)
    W3 = np.asarray(inputs["W3"], np.float32)
    b1 = np.asarray(inputs["b1"], np.float32)
    b2 = np.asarray(inputs["b2"], np.float32)
    b3 = np.asarray(inputs["b3"], np.float32)

    # pairing permutations: sort U features by bias so partition-paired
    # features share (nearly) one bias value
    pi = np.argsort(b1, kind="stable")      # L1 outputs
    sig = np.argsort(b2, kind="stable")     # L2 outputs
    # column placement: feature at output slot (mc*128 + p) is perm[2p + mc]
    pi_col = np.empty(D, np.int64)
    sig_col = np.empty(D, np.int64)
    pp = np.arange(P)
    for mc in range(2):
        pi_col[mc * P + pp] = pi[2 * pp + mc]
        sig_col[mc * P + pp] = sig[2 * pp + mc]
    b1s = 0.5 * (b1[pi[0::2]] + b1[pi[1::2]])   # [128]
    b2s = 0.5 * (b2[sig[0::2]] + b2[sig[1::2]])
    bs = np.stack([b1s, b2s], axis=1).astype(np.float32)

    # w1i[p, j, n] = 8*W1[2p+j, pi_col[n]]   (interleaved k for dma-transpose)
    w1i = (8.0 * W1[:, pi_col]).astype(f8).reshape(P, 2, D)
    # w1c[p, j, n] = 8*W1[j*128+p, pi_col[n]]  (chunked k for the chain)
    w1c = np.ascontiguousarray(
        (8.0 * W1[:, pi_col]).astype(f8).reshape(2, P, D).transpose(1, 0, 2))
    # w2i[p, j, n] = 8*W2[pi_col[j*128+p], sig_col[n]]
    w2p = (8.0 * W2[pi_col][:, sig_col]).astype(f8)
    w2i = np.ascontiguousarray(w2p.reshape(2, P, D).transpose(1, 0, 2))
    # w3m[p, j, m] = 8*W3[sig_col[j*128+p], m]
    w3p = (8.0 * W3[sig_col]).astype(f8)
    w3m = np.ascontiguousarray(w3p.reshape(2, P, D).transpose(1, 0, 2))
    # fp8 blob [p, j, 4, D]: w1i | w2i | w3m | w1c
    w8b = np.ascontiguousarray(np.stack([w1i, w2i, w3m, w1c], axis=2))
    # bf16 blob [1, D+P]: 8*b3 | ones
    hbo = np.zeros((1, D + P), np.float32)
    hbo[0, :D] = 8.0 * b3
    hbo[0, D:] = 1.0
    hbo = hbo.astype(bf)
    # f32 blob [P, P+2]: id32 | b1s | b2s
    f32b = np.concatenate([np.eye(P, dtype=np.float32), bs], axis=1)

    shared = dict(w8b=w8b, hbo=hbo, f32b=f32b)
    return h, shared


_CACHE = {}


def kernel(**inputs):
    from concourse.bass_utils import run_bass_kernel_spmd

    lat = np.ascontiguousarray(np.asarray(inputs["latents"], np.float32))
    h, shared = _host_inputs(inputs)
    b3 = np.asarray(inputs["b3"], np.float32)

    bf = ml_dtypes.bfloat16
    f8 = ml_dtypes.float8_e4m3
    lat8u = lat.astype(f8).view(np.uint16).view(bf)       # [B, 100, 128]
    latb = (lat + np.float32(h) * b3).astype(bf)          # [B, 100, 256]

    if h not in _CACHE:
        _CACHE[h] = _build(h)
    nc = _CACHE[h]

    in_maps = []
    for c in range(NCORES):
        m = dict(shared)
        m["lat8u"] = np.ascontiguousarray(lat8u[c * PB:(c + 1) * PB])
        m["latb"] = np.ascontiguousarray(latb[c * PB:(c + 1) * PB])
        in_maps.append(m)
    res = run_bass_kernel_spmd(nc, in_maps, list(range(NCORES)))

    out = np.empty((B, T, D), np.float32)
    out[:, 0] = lat[:, 0]
    for c in range(NCORES):
        sl = slice(c * PB, (c + 1) * PB)
        out[sl, 1:T_OBS + 1] = res.results[c]["out16"].astype(np.float32)
        ch = res.results[c]["outch"]          # [19, p, dc, b]
        out[sl, T_OBS + 1:] = ch.transpose(0, 3, 2, 1).reshape(
            KPRED - 1, P, D).transpose(1, 0, 2)
    out[:, 2] = lat[:, 1]
    return out

